# revision 1
# baseline (speedup 1.0000x reference)
"""Trainium2 Bass kernel for nn_Attention_57827439673725.

Dense transformer attention block (B=32, N=1024, C=1024, H=16, hd=64):
  qkv = x @ qkv_w + qkv_b ; q,k rms-normed (per head) and 2D-roped;
  out = softmax(q k^T / sqrt(hd)) v @ proj_w + proj_b

Pure data-parallel over batch across 8 NeuronCores (4 batches each).

Per-core design (v2 — cost-model-driven rewrite of the v1 baseline):
  phase A (per token tile): qkv matmul in fp8e4 DoubleRow mode as a 3-chain
        residual product (x_hi+x_lo)(W_hi+W_lo) minus the lo*lo term —
        x and W are hi/lo split on the host, W pre-scaled by 32 to clear
        fp8 subnormals (rms-norm makes q/k scale-invariant; 1/32 is folded
        into proj_w for the v path). DR packs two K-tiles per instruction
        at 0.5 cycles/row, so qkv costs 0.75x its bf16 cycles.
        the qkv psum is staged to SBUF bf16 by ONE ACT copy (releases the
        psum slot fast; the rope-congested DVE queue would hold it ~4us
        and stall the psum rotation). rms-norm squares/reduce/ln-bit-trick
        rsqrt Newton + rope all on DVE from SBUF (tables carry gamma and
        the rotate-half sign). q|k rope output [128, 2C] is transposed
        head-major by ONE DMA XBAR transpose per tile (no PE transposes,
        no PSUM psT, no copy-out): qkT[p, fb, tok], fb 0-7 = q, 8-15 = k.
        v lands via one ACT copy into v65 ([v_h | 1] per head, 65 cols).
  phase B (per head h, per j-tile): S^T = k q^T (bf16, K=64, row base
        64*(h%2) via tile_position); exp on ACT over the full [128 j,
        1024 i] psum -> P^T bf16; O in NATURAL layout: P^T block is the
        STATIONARY and v65 the 65-col moving operand, so O costs 65 moving
        cols per 128x65 output instead of 512 (halves O's PE time);
        the ones column accumulates the softmax denominator. PSUM zero
        regions allow only ONE pending accumulation group per 2KB bank, so
        psO is a [128,1024] 2-bank tile holding 4x65 cols per bank with a
        single start per bank (later first-writes read pending-zero, the
        rest accumulate). normalize = per-partition DVE reciprocal of the
        denominator columns + one broadcast-last multiply into attn4.
  phase C: attn4 tile -> DMA XBAR transpose -> attnT; proj bf16; psum ->
        SBUF on DVE; DMA out fp32.

Software pipeline (the cost model rewards LONG uninterrupted PE runs: the
p-state ramp halves PE speed for ~3us after every idle gap): phase B is a
slot pipeline where one (h, jt) slot = [weave chunk of ~12 fp8 matmuls
from C(b-1) / A(b+1), S, exp, O-octet delayed 2 slots]. The O delay
equals the psA-rotation distance, so O never stalls the in-order PE
stream. PSUM: 2x [128,1024] f32 (qkv/S/proj/v) + 2x [128,1024] f32 (O)
= 8 banks. SBUF: the SWDGE scratch ring is shrunk 16KB -> 2KB to fit the
double-buffered qkT (the weave keeps two batches of transposed q/k live).
"""

import os
import sys

import numpy as np

for _p in ("/opt/trn_rl_repo",):
    if os.path.isdir(_p) and _p not in sys.path:
        sys.path.insert(0, _p)

import ml_dtypes  # noqa: E402

import concourse.bass as bass  # noqa: E402
import concourse.mybir as mybir  # noqa: E402
import concourse.tile as tile  # noqa: E402
from concourse import bacc  # noqa: E402
from concourse.bass_utils import run_bass_kernel_spmd  # noqa: E402

BF16 = mybir.dt.bfloat16
F32 = mybir.dt.float32
F8 = mybir.dt.float8e4
NPBF16 = ml_dtypes.bfloat16
NPF8 = ml_dtypes.float8_e4m3fn

N_CORES = 8
B, N, C = 32, 1024, 1024
H, HD = 16, 64
BSH = B // N_CORES  # batches per core
NT = N // 128  # token tiles per batch
KT = C // 128  # k tiles over C
KP = KT // 2  # fp8 DoubleRow k-pair count
EPS = 1e-06
THETA = 10000.0
WSCALE = 32.0  # qkv_w prescale (clears fp8e4m3 subnormals)

MULT = mybir.AluOpType.mult
CHUNK = int(os.environ.get("KCHUNK", "12"))
PROLOG = int(os.environ.get("KPROLOG", "2"))
ADD = mybir.AluOpType.add
DR = mybir.MatmulPerfMode.DoubleRow


def _ap_with(ap: bass.AP, dims) -> bass.AP:
    return bass.AP(tensor=ap.tensor, offset=ap.offset, ap=dims)


def _bcast_mid(ap: bass.AP, n: int) -> bass.AP:
    """[P, F] -> [P, n, F] with a 0-step broadcast middle dim."""
    return _ap_with(ap, [ap.ap[0], [0, n], *ap.ap[1:]])


def _bcast_last(ap: bass.AP, n: int) -> bass.AP:
    """[P, F] -> [P, F, n] with a 0-step broadcast last dim."""
    return _ap_with(ap, [*ap.ap, [0, n]])


def _build_module(use_bias: bool, share_tabs: bool = False):
    nc = bacc.Bacc(
        "TRN2", target_bir_lowering=False, debug=False,
        dynamic_dma_scratch_size=2048,
    )

    xhi_d = nc.dram_tensor("xhi", [BSH, KT, 128, N], F8, kind="ExternalInput")
    xlo_d = nc.dram_tensor("xlo", [BSH, KT, 128, N], F8, kind="ExternalInput")
    whi_d = nc.dram_tensor("whi", [KT, 128, 3 * C], F8, kind="ExternalInput")
    wlo_d = nc.dram_tensor("wlo", [KT, 128, 3 * C], F8, kind="ExternalInput")
    wproj_d = nc.dram_tensor("wproj", [KT, 128, C], BF16, kind="ExternalInput")
    n_tab = 2 if share_tabs else 4
    tabs_d = nc.dram_tensor("tabs", [n_tab, NT, 128, HD], BF16, kind="ExternalInput")
    if use_bias:
        bq_d = nc.dram_tensor("bq", [3 * C], BF16, kind="ExternalInput")  # *WSCALE
        bp_d = nc.dram_tensor("bp", [C], BF16, kind="ExternalInput")
    out_d = nc.dram_tensor("out", [BSH, NT, 128, C], F32, kind="ExternalOutput")

    from contextlib import ExitStack

    with ExitStack() as ctx:
        tc = ctx.enter_context(tile.TileContext(nc))
        pool = lambda name, bufs, **kw: ctx.enter_context(  # noqa: E731
            tc.tile_pool(name=name, bufs=bufs, **kw)
        )
        cfg = dict(
            qk=2, v65=2, pt=2, at=2, rope=2, qs=1, stats=2, rec=2, outs=1,
            psA=2, psO=2, psU=5, psO2=2,
        )
        if use_bias or not share_tabs:
            # general fallback: correctness only; single-buffered qkT frees
            # the SBUF that the 4-table/bias tiles need
            cfg.update(dict(qk=1, at=1))
        for kv in os.environ.get("KBUFS", "").split(","):
            if kv:
                kk, vv = kv.split("=")
                cfg[kk] = int(vv)

        wpool = pool("weights", 1)
        cpool = pool("consts", 1)
        xtpool = pool("xt", 1)
        qkpool = pool("qkT", cfg["qk"])
        vpool = pool("v65", cfg["v65"])
        ptpool = pool("pt", cfg["pt"])
        a4pool = pool("attn4", 1)
        atpool = pool("attnT", cfg["at"])
        rpool = pool("rope", cfg["rope"])
        qspool = pool("qs", cfg["qs"])
        spool = pool("stats", cfg["stats"])
        recpool = pool("rec", cfg["rec"])
        opool = pool("outs", cfg["outs"])
        UNI = os.environ.get("KUNI", "1") == "1"
        if UNI:
            psU = pool("psU", cfg["psU"], space="PSUM")
            psO2 = pool("psO2", cfg["psO2"], space="PSUM")
            psDp = pool("psD", 1, space="PSUM")
        else:
            psA = pool("psA", cfg["psA"], space="PSUM")
            psO = pool("psO", cfg["psO"], space="PSUM")

        # ---- persistent weights / constants ----
        whi = wpool.tile([128, KT, 3 * C], F8, tag="whi")
        wlo = wpool.tile([128, KT, 3 * C], F8, tag="wlo")
        wproj = wpool.tile([128, KT, C], BF16, tag="wproj")

        tabs = cpool.tile([128, n_tab, NT, HD], BF16, tag="tabs")
        for i in range(n_tab):
            nc.sync.dma_start(
                out=tabs[:, i, :, :], in_=tabs_d[i].rearrange("t p d -> p t d")
            )
        if use_bias:
            bias_qkv = cpool.tile([128, 3 * C], BF16, tag="bq")
            bq_ap = bq_d[:]
            nc.sync.dma_start(
                out=bias_qkv[:, :], in_=_ap_with(bq_ap, [[0, 128], *bq_ap.ap])
            )
            bias_proj = cpool.tile([128, C], BF16, tag="bp")
            bp_ap = bp_d[:]
            nc.sync.dma_start(
                out=bias_proj[:, :], in_=_ap_with(bp_ap, [[0, 128], *bp_ap.ap])
            )
        eps_col = cpool.tile([128, 1], F32, tag="eps")
        nc.vector.memset(eps_col[:, :], EPS)
        ones_col = cpool.tile([128, 1], BF16, tag="ones1")
        nc.vector.memset(ones_col[:, :], 1.0)

        def load_x(b):
            xhi = xtpool.tile([128, KT, N], F8, tag="xhi", name="xhi")
            xlo = xtpool.tile([128, KT, N], F8, tag="xlo", name="xlo")
            nc.sync.dma_start(
                out=xhi[:, :, :], in_=xhi_d[b].rearrange("k p n -> p k n")
            )
            nc.sync.dma_start(
                out=xlo[:, :, :], in_=xlo_d[b].rearrange("k p n -> p k n")
            )
            return xhi, xlo

        def load_weights(b0):
            # first-consumed slices first: the opening q-chain needs only
            # whi k-pair 0 cols 0:512 and x tile 0 - land those in tiny DMAs
            # so PE starts ~2us in, then stream the rest
            nc.sync.dma_start(
                out=whi[:, 0:2, 0:512],
                in_=whi_d[0:2, :, 0:512].rearrange("k p n -> p k n"),
            )
            xhi = xtpool.tile([128, KT, N], F8, tag="xhi", name="xhi")
            xlo = xtpool.tile([128, KT, N], F8, tag="xlo", name="xlo")
            nc.sync.dma_start(
                out=xhi[:, :, 0:128], in_=xhi_d[b0, :, :, 0:128].rearrange("k p n -> p k n")
            )
            nc.sync.dma_start(
                out=whi[:, 2:, 0:512],
                in_=whi_d[2:, :, 0:512].rearrange("k p n -> p k n"),
            )
            nc.sync.dma_start(
                out=whi[:, :, 512:1024],
                in_=whi_d[:, :, 512:1024].rearrange("k p n -> p k n"),
            )
            nc.sync.dma_start(
                out=xhi[:, :, 128:], in_=xhi_d[b0, :, :, 128:].rearrange("k p n -> p k n")
            )
            nc.sync.dma_start(
                out=xlo[:, :, :], in_=xlo_d[b0].rearrange("k p n -> p k n")
            )
            xt0 = (xhi, xlo)
            nc.sync.dma_start(
                out=whi[:, :, 1024:],
                in_=whi_d[:, :, 1024:].rearrange("k p n -> p k n"),
            )
            nc.sync.dma_start(
                out=wlo[:, :, :], in_=wlo_d[:, :, :].rearrange("k p n -> p k n")
            )
            nc.sync.dma_start(
                out=wproj[:, :, :], in_=wproj_d[:, :, :].rearrange("k p n -> p k n")
            )
            return xt0

        def qkv_mm(ps, xt, col_lo, col_hi, chunk=None):
            """3-chain fp8 DR product into ps[:, 0:col_hi-col_lo].

            Generator when chunk is set: yields between groups of `chunk`
            matmuls so the caller can interleave other PE work.
            """
            xhi, xlo = xt
            w = col_hi - col_lo
            chains = [(xhi, whi), (xhi, wlo), (xlo, whi)]
            nch = len(chains)
            emitted = 0
            for ci, (xx, ww) in enumerate(chains):
                for kp in range(KP):
                    for half in range(0, w, 512):
                        nc.tensor.matmul(
                            ps[:, half : half + min(512, w - half)],
                            xx[:, 2 * kp : 2 * kp + 2, :],
                            ww[:, 2 * kp : 2 * kp + 2, col_lo + half : col_lo + half + min(512, w - half)],
                            start=(ci == 0 and kp == 0),
                            stop=(ci == nch - 1 and kp == KP - 1),
                            perf_mode=DR,
                        )
                        emitted += 1
                        if chunk and emitted % chunk == 0:
                            yield

        def qk_stage(ps, qi):
            """ONE copy psum -> SBUF bf16 so the psum slot frees fast."""
            stage = qspool.tile([128, 1024], BF16, tag="stage", name="stage", bufs=int(os.environ.get("KSBUF", "2")))
            if use_bias:
                nc.vector.scalar_tensor_tensor(
                    out=stage[:, :], in0=ps[:, :], scalar=1.0,
                    in1=bias_qkv[:, qi * 1024 : (qi + 1) * 1024],
                    op0=MULT, op1=ADD,
                )
            elif os.environ.get("KSTAGE", "act") == "act":
                nc.scalar.copy(stage[:, :], ps[:, :])
            elif os.environ.get("KSTAGE") == "split":
                nc.scalar.copy(stage[:, 0:512], ps[:, 0:512])
                nc.vector.tensor_copy(stage[:, 512:1024], ps[:, 512:1024])
            else:
                nc.vector.tensor_copy(stage[:, :], ps[:, :])
            return stage

        def qk_pipeline(stage_halves, qi, t, qrope):
            """rms norm + rope for q (qi=0) or k (qi=1)."""
            stage, halves = stage_halves
            src = stage

            # squares into the t1 scratch (reduce consumes it before rope
            # reuses t1)
            t1 = qspool.tile([128, 1024], BF16, tag="t1", name="t1")
            sq = t1[:, :]
            if halves:
                for hf in range(2):
                    nc.vector.tensor_mul(
                        sq.rearrange("p (a x) -> p a x", a=2)[:, hf, :],
                        halves[hf][:, :], halves[hf][:, :],
                    )
            else:
                nc.vector.tensor_mul(sq, src[:, :], src[:, :])
            sq3 = sq.rearrange("p (h d) -> p h d", d=HD)
            nc.vector.tensor_add(sq3[:, :, 0:32], sq3[:, :, 0:32], sq3[:, :, 32:64])
            var = spool.tile([128, H], F32, tag="var", name="var")
            nc.vector.reduce_sum(
                var[:, :], sq3[:, :, 0:32], axis=mybir.AxisListType.X
            )
            # rsqrt(var/HD + eps): DVE ln-bit-trick + ACT exp + one Newton step
            vv = spool.tile([128, H], F32, tag="vv", name="vv")
            nc.vector.tensor_scalar(
                out=vv[:, :], in0=var[:, :], scalar1=1.0 / HD, scalar2=EPS,
                op0=MULT, op1=ADD,
            )
            lnv = spool.tile([128, H], F32, tag="lnv", name="lnv")
            nc.vector.tensor_scalar(
                out=lnv[:, :], in0=vv[:, :].bitcast(mybir.dt.int32),
                scalar1=-1064866805, scalar2=8.2629582e-8,
                op0=ADD, op1=MULT,
            )
            if os.environ.get("KRSQRT", "dve") == "dve":
                # r0 = exp(-0.5 ln vv) via a second float bit-trick (no ACT):
                # float(bits) ~ A*(-0.5*lnv) + B, f32->i32 convert, bitcast
                r0i = spool.tile([128, H], mybir.dt.int32, tag="r0", name="r0i")
                nc.vector.tensor_scalar(
                    out=r0i[:, :], in0=lnv[:, :],
                    scalar1=-6051101.6, scalar2=1064866805.0,
                    op0=MULT, op1=ADD,
                )
                r0 = r0i[:, :].bitcast(mybir.dt.float32)
                rr = spool.tile([128, H], F32, tag="rr", name="rr")
                e2 = spool.tile([128, H], F32, tag="e2", name="e2")
                cur = r0
                for it in range(2):
                    nc.vector.tensor_mul(e2[:, :], cur, cur)
                    nc.vector.scalar_tensor_tensor(
                        out=e2[:, :], in0=e2[:, :], scalar=-0.5, in1=vv[:, :],
                        op0=MULT, op1=MULT,
                    )
                    nc.vector.scalar_tensor_tensor(
                        out=rr[:, :], in0=e2[:, :], scalar=1.5, in1=cur,
                        op0=ADD, op1=MULT,
                    )
                    cur = rr[:, :]
            else:
                r0 = spool.tile([128, H], F32, tag="r0", name="r0")
                nc.scalar.activation(
                    r0[:, :], lnv[:, :], mybir.ActivationFunctionType.Exp,
                    scale=-0.5,
                )
                e2 = spool.tile([128, H], F32, tag="e2", name="e2")
                nc.vector.tensor_mul(e2[:, :], r0[:, :], r0[:, :])
                nc.vector.scalar_tensor_tensor(
                    out=e2[:, :], in0=e2[:, :], scalar=-0.5, in1=vv[:, :],
                    op0=MULT, op1=MULT,
                )
                rr = spool.tile([128, H], F32, tag="rr", name="rr")
                nc.vector.scalar_tensor_tensor(
                    out=rr[:, :], in0=e2[:, :], scalar=1.5, in1=r0[:, :],
                    op0=ADD, op1=MULT,
                )

            # qs = src * rr (into the stage tile; in-place when staged)
            qs3 = src[:, :].rearrange("p (h d) -> p h d", d=HD)
            if halves:
                for hf in range(2):
                    nc.vector.tensor_mul(
                        qs3[:, hf * 8 : (hf + 1) * 8, :],
                        halves[hf][:, :].rearrange("p (h d) -> p h d", d=HD),
                        _bcast_last(rr[:, hf * 8 : (hf + 1) * 8], HD),
                    )
            else:
                nc.vector.tensor_mul(qs3, qs3, _bcast_last(rr[:, :], HD))

            # rope: out = qs * C + swap_halves(qs) * S (bf16, 2x DVE)
            qi_t = 0 if share_tabs else qi
            ctab = tabs[:, 2 * qi_t + 0, t, :]
            stab = tabs[:, 2 * qi_t + 1, t, :]
            t13 = t1[:, :].rearrange("p (h d) -> p h d", d=HD)
            eng = nc.gpsimd if os.environ.get("KROPE", "dve") == "pool" else nc.vector
            eng.tensor_mul(
                t13[:, :, 0:32], qs3[:, :, 32:64], _bcast_mid(stab[:, 0:32], H)
            )
            eng.tensor_mul(
                t13[:, :, 32:64], qs3[:, :, 0:32], _bcast_mid(stab[:, 32:64], H)
            )
            dst = qrope[:, qi * 1024 : (qi + 1) * 1024]
            eng.tensor_mul(
                dst.rearrange("p (h d) -> p h d", d=HD), qs3,
                _bcast_mid(ctab, H),
            )
            eng.tensor_add(dst, dst, t1[:, :])

        def a_step_gen(xt, qkT, v65, t, chunk=CHUNK):
            """one token tile of phase A as a generator: yields between PE
            chunks so the caller can interleave B-phase slots. Emits its
            own qkT transpose lazily on a late resumption (so the SP-queue
            DMA never waits long)."""
            qrope = rpool.tile([128, 2 * C], BF16, tag="qrope", name="qrope")
            xts = (xt[0][:, :, t * 128 : (t + 1) * 128],
                   xt[1][:, :, t * 128 : (t + 1) * 128])
            stages = []
            nostage = os.environ.get("KNOSTAGE", "0") == "1" and not use_bias
            for qi in range(2):
                stage = qspool.tile(
                    [128, 1024], BF16, tag="stage", name="stage",
                    bufs=int(os.environ.get("KSBUF", "2")),
                )
                halves = []
                for hf in range(2):
                    ph = psU.tile([128, 512], F32, tag="U", name="ps_qk")
                    lo = qi * 1024 + hf * 512
                    yield from qkv_mm(ph[:, :], xts, lo, lo + 512, chunk=chunk)
                    sl = slice(hf * 512, (hf + 1) * 512)
                    if nostage:
                        halves.append(ph)
                    elif use_bias:
                        nc.vector.scalar_tensor_tensor(
                            out=stage[:, sl], in0=ph[:, :], scalar=1.0,
                            in1=bias_qkv[:, lo : lo + 512],
                            op0=MULT, op1=ADD,
                        )
                    elif os.environ.get("KSTAGE", "act") == "act":
                        nc.scalar.copy(stage[:, sl], ph[:, :])
                    else:
                        nc.vector.tensor_copy(stage[:, sl], ph[:, :])
                stages.append((stage, halves))
                if qi == 1:
                    qk_pipeline(stages[0], 0, t, qrope)
                yield

            # v: 3-chain DR into two [128, 512] psums; copy into v65
            v3 = v65[:, t, :].rearrange("p (h e) -> p h e", e=65)
            for hf in range(2):
                psv = psU.tile([128, 512], F32, tag="U", name="psv")
                lo = 2048 + hf * 512
                yield from qkv_mm(psv[:, :], xts, lo, lo + 512, chunk=chunk)
                hs = slice(hf * 8, (hf + 1) * 8)
                pv3 = psv[:, :].rearrange("p (h d) -> p h d", d=64)
                if use_bias:
                    nc.vector.scalar_tensor_tensor(
                        out=v3[:, hs, 0:64], in0=pv3, scalar=1.0,
                        in1=bias_qkv[:, lo : lo + 512].rearrange(
                            "p (h d) -> p h d", d=64
                        ),
                        op0=MULT, op1=ADD,
                    )
                elif os.environ.get("KVCOPY", "act") == "act":
                    nc.scalar.copy(v3[:, hs, 0:64], pv3)
                else:
                    nc.vector.tensor_copy(v3[:, hs, 0:64], pv3)
            qk_pipeline(stages[1], 1, t, qrope)
            for _ in range(int(os.environ.get("KTDELAY", "1"))):
                yield
            # q|k rope [128, 2048] -> head-major qkT[:, 0:16, tslice]
            nc.sync.dma_start_transpose(
                qkT[:, :, t * 128 : (t + 1) * 128], qrope[:, :]
            )

        def v65_init(v65):
            v4 = v65[:, :, :].rearrange("p t (h e) -> p t h e", e=65)
            nc.vector.memset(v4[:, :, :, 64:65], 1.0)

        def b_phase(attn4, qkT, v65, weave_gen):
            """slot-pipelined phase B: per (h, jt) slot emit S+exp, the
            O-octet from 2 slots ago, and one weave chunk. O is delayed by
            the same distance the psA rotation already enforces, so it
            never stalls PE."""
            O_DELAY = int(os.environ.get("KODELAY", "1"))
            pending = []  # (h, jt, pt)
            psos = {}
            # batch-wide denominator bank: one accumulation group for the
            # whole bank (single start; later first-writes read pending-zero)
            psd = psDp.tile([128, H, NT], F32, tag="D", name="psd")

            def emit_o(h, jt, pt, ps_o_unused):
                if jt == 0:
                    psos[h] = psO2.tile([128, NT, 64], F32, tag="O", name="ps_o")
                ps_o = psos[h]
                vsl = v65[:, jt, h * 65 : h * 65 + 64]
                for ib in range(NT):
                    nc.tensor.matmul(
                        ps_o[:, ib, :],
                        pt[:, ib * 128 : (ib + 1) * 128],
                        vsl,
                        start=(jt == 0 and ib == 0),
                        stop=(jt == NT - 1 and ib == NT - 1),
                        skip_group_check=True,
                    )
                    nc.tensor.matmul(
                        psd[:, h, ib : ib + 1],
                        pt[:, ib * 128 : (ib + 1) * 128],
                        ones_col[:, :],
                        start=(h == 0 and jt == 0 and ib == 0),
                        stop=(h == H - 1 and jt == NT - 1 and ib == NT - 1),
                        skip_group_check=True,
                    )
                if jt == NT - 1:
                    rec = recpool.tile([128, NT], BF16, tag="rec", name="rec")
                    with nc.allow_low_precision("softmax denom recip bf16"):
                        nc.vector.reciprocal(rec[:, :], psd[:, h, :])
                    nc.vector.tensor_mul(
                        attn4[:, :, h, :], ps_o[:, :, :],
                        _bcast_last(rec[:, :], 64),
                    )
                    del psos[h]

            for h in range(H):
                base = 64 * (h % 2)
                fb = h // 2
                psl = slice(base, base + 64)
                for jt in range(NT):
                    if weave_gen is not None:
                        next(weave_gen, None)
                    pss = []
                    for ic in range(2):
                        ps_s = psU.tile([128, 512], F32, tag="U", name="ps_s")
                        nc.tensor.matmul(
                            ps_s[:, :],
                            qkT[psl, 8 + fb, jt * 128 : (jt + 1) * 128],
                            qkT[psl, fb, ic * 512 : (ic + 1) * 512],
                            start=True, stop=True,
                            tile_position=(base, 0),
                        )
                        pss.append(ps_s)
                    if len(pending) >= O_DELAY:
                        emit_o(*pending.pop(0))
                    pt = ptpool.tile([128, 1024], BF16, tag="pt", name="pt")
                    for ic in range(2):
                        nc.scalar.activation(
                            pt[:, ic * 512 : (ic + 1) * 512], pss[ic][:, :],
                            mybir.ActivationFunctionType.Exp, scale=0.125,
                        )
                    pending.append((h, jt, pt, None))
            for args in pending:
                emit_o(*args)

        def c_gen(attn4, b):
            """phase C as a generator (woven into the next batch's B slots):
            attn4 -> XBAR transpose -> attnT -> proj -> DMA out."""

            def transpose(t):
                att = atpool.tile([128, KT, 128], BF16, tag="at", name="attnT")
                nc.sync.dma_start_transpose(att[:, :, :], attn4[:, t, :, :])
                return att

            att = transpose(0)
            yield
            for t in range(NT):
                att_next = transpose(t + 1) if t + 1 < NT else None
                ostage = opool.tile([128, C], F32, tag="ostage", name="ostage")
                for half in range(2):
                    ps_p = psU.tile([128, 512], F32, tag="U", name="ps_p")
                    for k in range(KT):
                        nc.tensor.matmul(
                            ps_p[:, :],
                            att[:, k, :],
                            wproj[:, k, half * 512 : (half + 1) * 512],
                            start=(k == 0), stop=(k == KT - 1),
                        )
                    sl = slice(half * 512, (half + 1) * 512)
                    if use_bias:
                        nc.vector.tensor_add(
                            ostage[:, sl], ps_p[:, :], bias_proj[:, sl]
                        )
                    elif os.environ.get("KOCOPY", "dve") == "act":
                        nc.scalar.copy(ostage[:, sl], ps_p[:, :])
                    else:
                        nc.vector.tensor_copy(ostage[:, sl], ps_p[:, :])
                    yield
                nc.sync.dma_start(out=out_d[b, t], in_=ostage[:, :])
                yield
                att = att_next

        def alloc_ab():
            qkT = qkpool.tile([128, 2 * KT, N], BF16, tag="qkT", name="qkT")
            v65 = vpool.tile([128, NT, H * 65], BF16, tag="v65", name="v65")
            return qkT, v65

        reps = int(os.environ.get("KREPEAT", "1"))
        batches = [bb for _ in range(reps) for bb in range(BSH)]

        # prologue: weights + A(b0), two token-tile pipelines interleaved
        # so the rms/rope chain latency of one hides under the other's mms
        xt = load_weights(batches[0])
        tiles = alloc_ab()
        v65_init(tiles[1])
        from collections import deque

        _done = object()
        gens = [a_step_gen(xt, tiles[0], tiles[1], t) for t in range(NT)]
        active = deque(gens[:PROLOG])
        gi = PROLOG
        while active:
            g = active.popleft()
            if next(g, _done) is not _done:
                active.append(g)
            elif gi < NT:
                active.append(gens[gi])
                gi += 1

        from itertools import chain as _ichain

        prev_c = None  # (attn4, b) awaiting phase C
        for bi, b in enumerate(batches):
            qkT, v65 = tiles
            attn4 = a4pool.tile([128, NT, H, HD], BF16, tag="attn4", name="attn4")
            nxt = batches[bi + 1] if bi + 1 < len(batches) else None
            wparts = []
            if prev_c is not None:
                wparts.append(c_gen(*prev_c))
            if nxt is not None:
                xt2 = load_x(nxt)
                tiles2 = alloc_ab()
                v65_init(tiles2[1])

                _ck = int(os.environ.get("KCHUNK0", "5")) if prev_c is None else CHUNK

                def _weave(_xt=xt2, _tl=tiles2, _ck=_ck):
                    for t in range(NT):
                        yield from a_step_gen(_xt, _tl[0], _tl[1], t, chunk=_ck)

                wparts.append(_weave())
            if os.environ.get("KZIP", "0") == "1" and len(wparts) == 2:
                def _zip(parts):
                    from collections import deque as _dq
                    q = _dq(parts)
                    while q:
                        g = q.popleft()
                        try:
                            next(g)
                            q.append(g)
                        except StopIteration:
                            pass
                        yield
                wg = _zip(wparts)
            else:
                wg = _ichain(*wparts) if wparts else None
            b_phase(attn4, qkT, v65, wg)
            if wg is not None:
                for _ in wg:
                    pass
            prev_c = (attn4, b)
            if nxt is not None:
                xt, tiles = xt2, tiles2
        for _ in c_gen(*prev_c):
            pass

    nc.compile()
    return nc


_NC = {}


def _get_nc(use_bias: bool = False, share_tabs: bool = False):
    key = (use_bias, share_tabs)
    if key not in _NC:
        _NC[key] = _build_module(use_bias, share_tabs)
    return _NC[key]


def _rope_tables():
    """cos/sin tables exactly as reference.rope_tables, in float32."""
    grid = int(np.sqrt(N))
    half = HD // 2
    freqs = (1.0 / THETA ** (np.arange(0, half, 2, dtype=np.float32) / half)).astype(
        np.float32
    )
    freqs = np.concatenate([freqs, freqs], axis=0)
    t = np.arange(grid, dtype=np.float32)
    f = np.outer(t, freqs).astype(np.float32)
    fh = np.broadcast_to(f[:, None, :], (grid, grid, half))
    fw = np.broadcast_to(f[None, :, :], (grid, grid, half))
    full = np.concatenate([fh, fw], axis=-1).reshape(-1, HD).astype(np.float32)
    return np.cos(full).astype(np.float32), np.sin(full).astype(np.float32)


def _make_inputs(x, qkv_w, qkv_b, proj_w, proj_b, q_gamma, k_gamma,
                 use_bias=False, share_tabs=False):
    cos, sin = _rope_tables()
    sgn = np.where(np.arange(HD) < HD // 2, -1.0, 1.0).astype(np.float32)
    swap = (np.arange(HD) + HD // 2) % HD

    def fold(gamma):
        c = (cos * gamma[None, :]).astype(np.float32)
        s = (sin * sgn[None, :] * gamma[swap][None, :]).astype(np.float32)
        return c, s

    cq, sq = fold(q_gamma.astype(np.float32))
    if share_tabs:
        stack = [cq, sq]
    else:
        ck, sk = fold(k_gamma.astype(np.float32))
        stack = [cq, sq, ck, sk]
    tabs = np.stack(stack, axis=0).reshape(len(stack), NT, 128, HD).astype(NPBF16)

    ws = (qkv_w.astype(np.float32) * WSCALE).reshape(KT, 128, 3 * C)
    whi = np.ascontiguousarray(ws).astype(NPF8)
    wlo = (ws - whi.astype(np.float32)).astype(NPF8)
    wproj_h = np.ascontiguousarray(
        (proj_w.astype(np.float32) / WSCALE).reshape(KT, 128, C)
    ).astype(NPBF16)

    in_maps = []
    for c in range(N_CORES):
        xc = x[c * BSH : (c + 1) * BSH].astype(np.float32)  # [BSH, N, C]
        xt = np.ascontiguousarray(xc.transpose(0, 2, 1)).reshape(BSH, KT, 128, N)
        xhi = xt.astype(NPF8)
        xlo = (xt - xhi.astype(np.float32)).astype(NPF8)
        m = {
            "xhi": xhi,
            "xlo": xlo,
            "whi": whi,
            "wlo": wlo,
            "wproj": wproj_h,
            "tabs": tabs,
        }
        if use_bias:
            m["bq"] = (qkv_b.astype(np.float32) * WSCALE).astype(NPBF16)
            m["bp"] = proj_b.astype(np.float32).astype(NPBF16)
        in_maps.append(m)
    return in_maps


def _run(in_maps, use_bias=False, share_tabs=False, trace=False, **kwargs):
    nc = _get_nc(use_bias, share_tabs)
    return run_bass_kernel_spmd(
        nc, in_maps, core_ids=list(range(N_CORES)), trace=trace, **kwargs
    )


def kernel(x, qkv_w, qkv_b, proj_w, proj_b, q_gamma, k_gamma):
    x = np.asarray(x)
    qkv_b = np.asarray(qkv_b)
    proj_b = np.asarray(proj_b)
    use_bias = bool(np.any(qkv_b != 0) or np.any(proj_b != 0))
    q_gamma = np.asarray(q_gamma)
    k_gamma = np.asarray(k_gamma)
    share_tabs = bool(np.array_equal(q_gamma, k_gamma))
    in_maps = _make_inputs(
        x, np.asarray(qkv_w), qkv_b, np.asarray(proj_w), proj_b,
        q_gamma, k_gamma, use_bias=use_bias, share_tabs=share_tabs,
    )
    res = _run(in_maps, use_bias=use_bias, share_tabs=share_tabs)
    outs = [res.results[c]["out"].reshape(BSH, NT * 128, C) for c in range(N_CORES)]
    return np.concatenate(outs, axis=0).astype(np.float32)



# revision 7
# speedup vs baseline: 1.1102x; 1.1102x over previous
"""Trainium2 Bass kernel for nn_Attention_57827439673725.

Dense transformer attention block (B=32, N=1024, C=1024, H=16, hd=64):
  qkv = x @ qkv_w + qkv_b ; q,k rms-normed (per head) and 2D-roped;
  out = softmax(q k^T / sqrt(hd)) v @ proj_w + proj_b

Pure data-parallel over batch across 8 NeuronCores (4 batches each).

v4 design (fp8 attention core; exp split across ACT+DVE; rope tail on Pool):
  phase A (per token tile): qkv = 2-chain fp8e4 DoubleRow product
        xhi*(whi+wlo) (the xlo*whi chain of v2/v3 is dropped: its error is
        attention-averaged to ~0.2% final; halves the x DMA). q/k psums are
        ACT-copied to SBUF bf16, rms-normed + roped on DVE; the two final
        rope ops run on the otherwise-idle Pool engine and emit fp8e4
        directly (q scaled x8 in the host tables for fp8 range). The fp8
        q|k tile [128, 2048] is XBAR-transposed as uint16 PAIRS: head h
        lands at partitions 32*(h%4)..+32 with the (d, d+1) hd-pair in the
        two bytes of each u16 - exactly the [32, 2(pair), tok] layout that
        DoubleRow needs to contract hd=64 as 2x32 partition-tiles at 0.5
        cycles/row. v lands via one ACT copy into fp8 v65 ([v_h|1] per
        head).
  phase B (per head h, per j-tile): S^T = k q^T in fp8 DR (half the bf16
        cost); exp over the [128 j, 1024 i] psum emits P^T fp8e4 into a
        [128, 2(jt), 1024] pair tile, alternating engines: ACT slots use
        table exp (scale 1/64, bias -2 so max fp8 448 is never hit), DVE
        slots use a 1-op Schraudolph bit-trick (bits = psum*0.18034 +
        32.467 -> saturating round-to-nearest uint8 = the fp8e4m3 bit
        pattern; low end saturates to +0.0, exactly softmax semantics).
        O accumulates per jt-PAIR in DR mode: stationary P^T pair
        [128, 2, 128], moving v65 pair [128, 2, 64] -> 32.5 cycles per
        128x64 output (4x cheaper than v3); the softmax denominator runs
        as a parallel DR matmul against an fp8 ones pair. normalize =
        per-partition DVE reciprocal + one broadcast-last multiply.
  phase C: attn4 -> DMA XBAR transpose -> attnT; proj bf16; psum -> SBUF;
        DMA out fp32.

Slot pipeline: same weave as v2/v3 (C(b-1) + A(b+1) generators yield
between chunks of PE work inside B(b) slots) - phase B's PE work is now
tiny (~360ns/slot) vs the exp engines (~1.1us/slot), so the weave is what
keeps PE busy and ramped. PSUM: one shared pool of 3x [128,1024] f32
(2 banks each: S slots, qkv tiles, proj tiles) + psO [128, NT, 64] +
psD [128, H, NT] = 8 banks.
"""

import os
import sys

import numpy as np

for _p in ("/opt/trn_rl_repo",):
    if os.path.isdir(_p) and _p not in sys.path:
        sys.path.insert(0, _p)

import ml_dtypes  # noqa: E402

import concourse.bass as bass  # noqa: E402
import concourse.mybir as mybir  # noqa: E402
import concourse.tile as tile  # noqa: E402
from concourse import bacc  # noqa: E402
from concourse.bass_utils import run_bass_kernel_spmd  # noqa: E402

BF16 = mybir.dt.bfloat16
F32 = mybir.dt.float32
F8 = mybir.dt.float8e4
U16 = mybir.dt.uint16
U8 = mybir.dt.uint8
NPBF16 = ml_dtypes.bfloat16
NPF8 = ml_dtypes.float8_e4m3fn

N_CORES = 8
B, N, C = 32, 1024, 1024
H, HD = 16, 64
BSH = B // N_CORES  # batches per core
NT = N // 128  # token tiles per batch
KT = C // 128  # k tiles over C
KP = KT // 2  # fp8 DoubleRow k-pair count
EPS = 1e-06
THETA = 10000.0
WSCALE = 32.0  # qkv_w prescale (clears fp8e4m3 subnormals)
QSCALE = 8.0  # q rope-table prescale (fp8 range for S operands)

# exp: logit L = S_psum/64 (q carries x8); P = exp(L - DELTA) in fp8e4m3.
DELTA = 2.0
EXP_S1 = 8.0 / (64.0 * np.log(2.0))  # 0.18033688
EXP_S2 = 56.0 - 0.45 - DELTA * 8.0 / np.log(2.0)  # 32.467 (c=-0.45 tuned)

MULT = mybir.AluOpType.mult
ADD = mybir.AluOpType.add
DR = mybir.MatmulPerfMode.DoubleRow
CHUNK = int(os.environ.get("KCHUNK", "6"))
PROLOG = int(os.environ.get("KPROLOG", "2"))
EXPDVE = int(os.environ.get("KEXPDVE", "3"))  # of every 8 slots, this many on DVE
O_DELAY = int(os.environ.get("KODELAY", "1"))  # in jt-pairs
KPOOL = int(os.environ.get("KPOOL", "3"))  # 1: rope add on Pool; 2: t13 muls too


def _ap_with(ap: bass.AP, dims) -> bass.AP:
    return bass.AP(tensor=ap.tensor, offset=ap.offset, ap=dims)


def _bcast_mid(ap: bass.AP, n: int) -> bass.AP:
    """[P, F] -> [P, n, F] with a 0-step broadcast middle dim."""
    return _ap_with(ap, [ap.ap[0], [0, n], *ap.ap[1:]])


def _bcast_last(ap: bass.AP, n: int) -> bass.AP:
    """[P, F] -> [P, F, n] with a 0-step broadcast last dim."""
    return _ap_with(ap, [*ap.ap, [0, n]])


def _build_module(use_bias: bool):
    nc = bacc.Bacc(
        "TRN2", target_bir_lowering=False, debug=False,
        dynamic_dma_scratch_size=2048,
    )

    xhi_d = nc.dram_tensor("xhi", [BSH, KT, 128, N], F8, kind="ExternalInput")
    whi_d = nc.dram_tensor("whi", [KT, 128, 3 * C], F8, kind="ExternalInput")
    wlo_d = nc.dram_tensor("wlo", [KT, 128, 3 * C], F8, kind="ExternalInput")
    wproj_d = nc.dram_tensor("wproj", [KT, 128, C], BF16, kind="ExternalInput")
    tabs_d = nc.dram_tensor("tabs", [4, NT, 128, HD], BF16, kind="ExternalInput")
    if use_bias:
        bq_d = nc.dram_tensor("bq", [3 * C], BF16, kind="ExternalInput")  # *WSCALE
        bp_d = nc.dram_tensor("bp", [C], BF16, kind="ExternalInput")
    out_d = nc.dram_tensor("out", [BSH, NT, 128, C], F32, kind="ExternalOutput")

    from contextlib import ExitStack

    with ExitStack() as ctx:
        tc = ctx.enter_context(tile.TileContext(nc))
        pool = lambda name, bufs, **kw: ctx.enter_context(  # noqa: E731
            tc.tile_pool(name=name, bufs=bufs, **kw)
        )
        cfg = dict(
            qk=2, v65=2, pt=3, at=2, rope=2, qs=2, t1=2, t2=2, stats=2,
            rec=2, outs=2, psA=3, psO=1,
        )
        for kv in os.environ.get("KBUFS", "").split(","):
            if kv:
                kk, vv = kv.split("=")
                cfg[kk] = int(vv)

        wpool = pool("weights", 1)
        cpool = pool("consts", 1)
        xtpool = pool("xt", 1)
        qkpool = pool("qkT", cfg["qk"])
        vpool = pool("v65", cfg["v65"])
        ptpool = pool("pt", cfg["pt"])
        a4pool = pool("attn4", 1)
        atpool = pool("attnT", cfg["at"])
        rpool = pool("rope", cfg["rope"])
        qspool = pool("qs", cfg["qs"])
        t1pool = pool("t1", cfg["t1"])
        t2pool = pool("t2", cfg["t2"])
        spool = pool("stats", cfg["stats"])
        recpool = pool("rec", cfg["rec"])
        opool = pool("outs", cfg["outs"])
        psA = pool("psA", cfg["psA"], space="PSUM")
        psO2 = pool("psO", cfg["psO"], space="PSUM")
        psDp = pool("psD", 1, space="PSUM")

        # ---- persistent weights / constants ----
        whi = wpool.tile([128, KT, 3 * C], F8, tag="whi")
        wlo = wpool.tile([128, KT, 3 * C], F8, tag="wlo")
        wproj = wpool.tile([128, KT, C], BF16, tag="wproj")

        tabs = cpool.tile([128, 4, NT, HD], BF16, tag="tabs")
        for i in range(4):
            nc.sync.dma_start(
                out=tabs[:, i, :, :], in_=tabs_d[i].rearrange("t p d -> p t d")
            )
        if use_bias:
            bias_qkv = cpool.tile([128, 3 * C], BF16, tag="bq")
            bq_ap = bq_d[:]
            nc.sync.dma_start(
                out=bias_qkv[:, :], in_=_ap_with(bq_ap, [[0, 128], *bq_ap.ap])
            )
            bias_proj = cpool.tile([128, C], BF16, tag="bp")
            bp_ap = bp_d[:]
            nc.sync.dma_start(
                out=bias_proj[:, :], in_=_ap_with(bp_ap, [[0, 128], *bp_ap.ap])
            )
        ones2 = cpool.tile([128, 2, 1], F8, tag="ones2")
        nc.vector.memset(ones2[:, :, :], 1.0)
        negd = cpool.tile([128, 1], F32, tag="negd")
        nc.vector.memset(negd[:, :], -DELTA)

        def load_x(b):
            xhi = xtpool.tile([128, KT, N], F8, tag="xhi", name="xhi")
            nc.sync.dma_start(
                out=xhi[:, :, :], in_=xhi_d[b].rearrange("k p n -> p k n")
            )
            return xhi

        def load_weights(b0):
            # first-consumed slices first: the opening q-chain needs only
            # whi/wlo k-pair cols 0:512 and x tile 0 - land those in small
            # DMAs so PE starts early, then stream the rest
            nc.sync.dma_start(
                out=whi[:, :, 0:512],
                in_=whi_d[:, :, 0:512].rearrange("k p n -> p k n"),
            )
            xhi = xtpool.tile([128, KT, N], F8, tag="xhi", name="xhi")
            nc.sync.dma_start(
                out=xhi[:, :, 0:128], in_=xhi_d[b0, :, :, 0:128].rearrange("k p n -> p k n")
            )
            nc.sync.dma_start(
                out=wlo[:, :, 0:512],
                in_=wlo_d[:, :, 0:512].rearrange("k p n -> p k n"),
            )
            nc.sync.dma_start(
                out=whi[:, :, 512:1024],
                in_=whi_d[:, :, 512:1024].rearrange("k p n -> p k n"),
            )
            nc.sync.dma_start(
                out=wlo[:, :, 512:1024],
                in_=wlo_d[:, :, 512:1024].rearrange("k p n -> p k n"),
            )
            nc.sync.dma_start(
                out=xhi[:, :, 128:], in_=xhi_d[b0, :, :, 128:].rearrange("k p n -> p k n")
            )
            nc.sync.dma_start(
                out=whi[:, :, 1024:],
                in_=whi_d[:, :, 1024:].rearrange("k p n -> p k n"),
            )
            nc.sync.dma_start(
                out=wlo[:, :, 1024:],
                in_=wlo_d[:, :, 1024:].rearrange("k p n -> p k n"),
            )
            nc.sync.dma_start(
                out=wproj[:, :, :], in_=wproj_d[:, :, :].rearrange("k p n -> p k n")
            )
            return xhi

        def qkv_mm(ps, xt, col_lo, chunk=None):
            """2-chain fp8 DR product into ps[:, 0:1024].

            Generator when chunk is set: yields between groups of `chunk`
            matmuls so the caller can interleave other PE work.
            """
            chains = [whi, wlo]
            nch = len(chains)
            emitted = 0
            for ci, ww in enumerate(chains):
                for kp in range(KP):
                    for half in range(2):
                        nc.tensor.matmul(
                            ps[:, half * 512 : (half + 1) * 512],
                            xt[:, 2 * kp : 2 * kp + 2, :],
                            ww[:, 2 * kp : 2 * kp + 2,
                               col_lo + half * 512 : col_lo + (half + 1) * 512],
                            start=(ci == 0 and kp == 0),
                            stop=(ci == nch - 1 and kp == KP - 1),
                            perf_mode=DR,
                        )
                        emitted += 1
                        if chunk and emitted % chunk == 0:
                            yield

        def qk_pipeline(stage, qi, t, qrope):
            """rms norm + rope for q (qi=0) or k (qi=1). stage is the bf16
            SBUF copy of the qkv psum. Writes fp8 into qrope[:, qi*1024:]."""
            src = stage
            t1 = t1pool.tile([128, 1024], BF16, tag="t1", name="t1")
            sq = t1[:, :]
            nc.vector.tensor_mul(sq, src[:, :], src[:, :])
            sq3 = sq.rearrange("p (h d) -> p h d", d=HD)
            nc.vector.tensor_add(sq3[:, :, 0:32], sq3[:, :, 0:32], sq3[:, :, 32:64])
            var = spool.tile([128, H], F32, tag="var", name="var")
            nc.vector.reduce_sum(
                var[:, :], sq3[:, :, 0:32], axis=mybir.AxisListType.X
            )
            # rsqrt(var/HD + eps): DVE ln-bit-trick + exp-bit-trick + Newton
            vv = spool.tile([128, H], F32, tag="vv", name="vv")
            nc.vector.tensor_scalar(
                out=vv[:, :], in0=var[:, :], scalar1=1.0 / HD, scalar2=EPS,
                op0=MULT, op1=ADD,
            )
            lnv = spool.tile([128, H], F32, tag="lnv", name="lnv")
            nc.vector.tensor_scalar(
                out=lnv[:, :], in0=vv[:, :].bitcast(mybir.dt.int32),
                scalar1=-1064866805, scalar2=8.2629582e-8,
                op0=ADD, op1=MULT,
            )
            r0i = spool.tile([128, H], mybir.dt.int32, tag="r0", name="r0i")
            nc.vector.tensor_scalar(
                out=r0i[:, :], in0=lnv[:, :],
                scalar1=-6051101.6, scalar2=1064866805.0,
                op0=MULT, op1=ADD,
            )
            r0 = r0i[:, :].bitcast(mybir.dt.float32)
            rr = spool.tile([128, H], F32, tag="rr", name="rr")
            e2 = spool.tile([128, H], F32, tag="e2", name="e2")
            cur = r0
            for _ in range(2):
                nc.vector.tensor_mul(e2[:, :], cur, cur)
                nc.vector.scalar_tensor_tensor(
                    out=e2[:, :], in0=e2[:, :], scalar=-0.5, in1=vv[:, :],
                    op0=MULT, op1=MULT,
                )
                nc.vector.scalar_tensor_tensor(
                    out=rr[:, :], in0=e2[:, :], scalar=1.5, in1=cur,
                    op0=ADD, op1=MULT,
                )
                cur = rr[:, :]

            # qs = src * rr (in-place into stage)
            qs3 = src[:, :].rearrange("p (h d) -> p h d", d=HD)
            nc.vector.tensor_mul(qs3, qs3, _bcast_last(rr[:, :], HD))

            # rope: out = qs*C + swap_halves(qs)*S; final ops emit fp8
            ctab = tabs[:, 2 * qi + 0, t, :]
            stab = tabs[:, 2 * qi + 1, t, :]
            t13 = t1[:, :].rearrange("p (h d) -> p h d", d=HD)
            eng13 = nc.gpsimd if (KPOOL & 2) else nc.vector
            eng13.tensor_mul(
                t13[:, :, 0:32], qs3[:, :, 32:64], _bcast_mid(stab[:, 0:32], H)
            )
            eng13.tensor_mul(
                t13[:, :, 32:64], qs3[:, :, 0:32], _bcast_mid(stab[:, 32:64], H)
            )
            t2 = t2pool.tile([128, 1024], BF16, tag="t2", name="t2")
            nc.vector.tensor_mul(
                t2[:, :].rearrange("p (h d) -> p h d", d=HD), qs3,
                _bcast_mid(ctab, H),
            )
            dst = qrope[:, qi * 1024 : (qi + 1) * 1024]
            engadd = nc.gpsimd if (KPOOL & 1) else nc.vector
            engadd.tensor_add(dst, t2[:, :], t1[:, :])

        def a_step_gen(xt, qkT2, v65, t, chunk=CHUNK):
            """one token tile of phase A as a generator: yields between PE
            chunks so the caller can interleave B-phase slots."""
            qrope = rpool.tile([128, 2 * C], F8, tag="qrope", name="qrope")
            xts = xt[:, :, t * 128 : (t + 1) * 128]
            stages = []
            for qi in range(2):
                ph = psA.tile([128, 1024], F32, tag="A", name="ps_qk")
                yield from qkv_mm(ph[:, :], xts, qi * 1024, chunk=chunk)
                stage = qspool.tile([128, 1024], BF16, tag="stage", name="stage")
                if use_bias:
                    nc.vector.scalar_tensor_tensor(
                        out=stage[:, :], in0=ph[:, :], scalar=1.0,
                        in1=bias_qkv[:, qi * 1024 : (qi + 1) * 1024],
                        op0=MULT, op1=ADD,
                    )
                else:
                    nc.scalar.copy(stage[:, :], ph[:, :])
                stages.append(stage)
                if qi == 1:
                    qk_pipeline(stages[0], 0, t, qrope)
                yield

            # v: 2-chain DR into one [128, 1024] psum; one ACT copy into v65
            psv = psA.tile([128, 1024], F32, tag="A", name="psv")
            yield from qkv_mm(psv[:, :], xts, 2048, chunk=chunk)
            v3 = v65[:, t, :].rearrange("p (h e) -> p h e", e=65)
            pv3 = psv[:, :].rearrange("p (h d) -> p h d", d=64)
            if use_bias:
                nc.vector.scalar_tensor_tensor(
                    out=v3[:, :, 0:64], in0=pv3, scalar=1.0,
                    in1=bias_qkv[:, 2048:3072].rearrange("p (h d) -> p h d", d=64),
                    op0=MULT, op1=ADD,
                )
            else:
                nc.scalar.copy(v3[:, :, 0:64], pv3)
            qk_pipeline(stages[1], 1, t, qrope)
            yield
            # q|k rope fp8 [128, 2048] -> u16-pair transpose -> qkT2
            nc.sync.dma_start_transpose(
                qkT2[:, :, t * 128 : (t + 1) * 128], qrope[:, :].bitcast(U16)
            )

        def v65_init(v65):
            v4 = v65[:, :, :].rearrange("p t (h e) -> p t h e", e=65)
            nc.vector.memset(v4[:, :, :, 64:65], 1.0)

        def b_phase(attn4, qkT2, v65, weave_gen):
            """slot-pipelined phase B: per (h, jt) slot emit S(DR)+exp; per
            jt-pair emit the delayed O-DR octet."""
            pending = []  # (h, jp, pt)
            psos = {}
            psd = psDp.tile([128, H, NT], F32, tag="D", name="psd")
            slot = 0

            def emit_o(h, jp, pt):
                if jp == 0:
                    psos[h] = psO2.tile([128, NT, 64], F32, tag="O", name="ps_o")
                ps_o = psos[h]
                vsl = v65[:, 2 * jp : 2 * jp + 2, h * 65 : h * 65 + 64]
                for ib in range(NT):
                    ptb = pt[:, :, ib * 128 : (ib + 1) * 128]
                    nc.tensor.matmul(
                        ps_o[:, ib, :], ptb, vsl,
                        start=(jp == 0 and ib == 0),
                        stop=(jp == 3 and ib == NT - 1),
                        perf_mode=DR,
                        skip_group_check=True,
                    )
                    nc.tensor.matmul(
                        psd[:, h, ib : ib + 1], ptb, ones2[:, :, :],
                        start=(h == 0 and jp == 0 and ib == 0),
                        stop=(h == H - 1 and jp == 3 and ib == NT - 1),
                        perf_mode=DR,
                        skip_group_check=True,
                    )
                if jp == 3:
                    rec = recpool.tile([128, NT], BF16, tag="rec", name="rec")
                    with nc.allow_low_precision("softmax denom recip bf16"):
                        nc.vector.reciprocal(rec[:, :], psd[:, h, :])
                    nc.vector.tensor_mul(
                        attn4[:, :, h, :], psos[h][:, :, :],
                        _bcast_last(rec[:, :], 64),
                    )
                    del psos[h]

            pt = None
            for h in range(H):
                pb = 32 * (h % 4)
                fbq = h // 4
                fbk = 4 + h // 4
                for jt in range(NT):
                    if weave_gen is not None:
                        next(weave_gen, None)
                    if jt % 2 == 0:
                        pt = ptpool.tile([128, 2, 1024], F8, tag="pt", name="pt")
                    ps_s = psA.tile([128, 1024], F32, tag="A", name="ps_s")
                    lk = qkT2[pb : pb + 32, fbk, jt * 128 : (jt + 1) * 128] \
                        .bitcast(F8).rearrange("p (t b) -> p b t", b=2)
                    for ic in range(2):
                        rq = qkT2[pb : pb + 32, fbq, ic * 512 : (ic + 1) * 512] \
                            .bitcast(F8).rearrange("p (i b) -> p b i", b=2)
                        nc.tensor.matmul(
                            ps_s[:, ic * 512 : (ic + 1) * 512],
                            lk, rq, start=True, stop=True, perf_mode=DR,
                            tile_position=(pb, 0),
                        )
                    if len(pending) >= O_DELAY:
                        emit_o(*pending.pop(0))
                    dst = pt[:, jt % 2, :]
                    if (slot * EXPDVE) % 8 < EXPDVE:
                        nc.vector.tensor_scalar(
                            out=dst.bitcast(U8), in0=ps_s[:, :],
                            scalar1=EXP_S1, scalar2=EXP_S2,
                            op0=MULT, op1=ADD,
                        )
                    else:
                        nc.scalar.activation(
                            dst, ps_s[:, :],
                            mybir.ActivationFunctionType.Exp,
                            scale=1.0 / 64.0, bias=negd[:, :],
                        )
                    slot += 1
                    if jt % 2 == 1:
                        pending.append((h, jt // 2, pt))
            for args in pending:
                emit_o(*args)

        def c_gen(attn4, b):
            """phase C as a generator (woven into the next batch's B slots):
            attn4 -> XBAR transpose -> attnT -> proj -> DMA out."""

            def transpose(t):
                att = atpool.tile([128, KT, 128], BF16, tag="at", name="attnT")
                nc.sync.dma_start_transpose(att[:, :, :], attn4[:, t, :, :])
                return att

            att = transpose(0)
            yield
            for t in range(NT):
                att_next = transpose(t + 1) if t + 1 < NT else None
                ps_p = psA.tile([128, 1024], F32, tag="A", name="ps_p")
                for half in range(2):
                    for k in range(KT):
                        nc.tensor.matmul(
                            ps_p[:, half * 512 : (half + 1) * 512],
                            att[:, k, :],
                            wproj[:, k, half * 512 : (half + 1) * 512],
                            start=(k == 0), stop=(k == KT - 1),
                        )
                    yield
                ostage = opool.tile([128, C], F32, tag="ostage", name="ostage")
                if use_bias:
                    nc.vector.tensor_add(
                        ostage[:, :], ps_p[:, :], bias_proj[:, :]
                    )
                elif os.environ.get("KOCOPY", "split") == "act":
                    nc.scalar.copy(ostage[:, :], ps_p[:, :])
                elif os.environ.get("KOCOPY", "split") == "dve":
                    nc.vector.tensor_copy(ostage[:, :], ps_p[:, :])
                else:
                    nc.scalar.copy(ostage[:, 0:512], ps_p[:, 0:512])
                    nc.vector.tensor_copy(ostage[:, 512:1024], ps_p[:, 512:1024])
                nc.sync.dma_start(out=out_d[b, t], in_=ostage[:, :])
                yield
                att = att_next

        def alloc_ab():
            qkT2 = qkpool.tile([128, KT, N], U16, tag="qkT2", name="qkT2")
            v65 = vpool.tile([128, NT, H * 65], F8, tag="v65", name="v65")
            return qkT2, v65

        reps = int(os.environ.get("KREPEAT", "1"))
        batches = [bb for _ in range(reps) for bb in range(BSH)]

        # prologue: weights + A(b0), two token-tile pipelines interleaved
        xt = load_weights(batches[0])
        tiles = alloc_ab()
        v65_init(tiles[1])
        from collections import deque

        _done = object()
        gens = [a_step_gen(xt, tiles[0], tiles[1], t) for t in range(NT)]
        active = deque(gens[:PROLOG])
        gi = PROLOG
        while active:
            g = active.popleft()
            if next(g, _done) is not _done:
                active.append(g)
            elif gi < NT:
                active.append(gens[gi])
                gi += 1

        from itertools import chain as _ichain

        prev_c = None  # (attn4, b) awaiting phase C
        for bi, b in enumerate(batches):
            qkT2, v65 = tiles
            attn4 = a4pool.tile([128, NT, H, HD], BF16, tag="attn4", name="attn4")
            nxt = batches[bi + 1] if bi + 1 < len(batches) else None
            wparts = []
            if prev_c is not None:
                wparts.append(c_gen(*prev_c))
            if nxt is not None:
                xt2 = load_x(nxt)
                tiles2 = alloc_ab()
                v65_init(tiles2[1])

                _ck = int(os.environ.get("KCHUNK0", "4")) if prev_c is None else CHUNK

                def _weave(_xt=xt2, _tl=tiles2, _ck=_ck):
                    for t in range(NT):
                        yield from a_step_gen(_xt, _tl[0], _tl[1], t, chunk=_ck)

                wparts.append(_weave())
            wg = _ichain(*wparts) if wparts else None
            b_phase(attn4, qkT2, v65, wg)
            if wg is not None:
                for _ in wg:
                    pass
            prev_c = (attn4, b)
            if nxt is not None:
                xt, tiles = xt2, tiles2
        for _ in c_gen(*prev_c):
            pass

    nc.compile()
    return nc


_NC = {}


def _get_nc(use_bias: bool = False, share_tabs: bool = False):
    key = bool(use_bias)
    if key not in _NC:
        _NC[key] = _build_module(key)
    return _NC[key]


def _rope_tables():
    """cos/sin tables exactly as reference.rope_tables, in float32."""
    grid = int(np.sqrt(N))
    half = HD // 2
    freqs = (1.0 / THETA ** (np.arange(0, half, 2, dtype=np.float32) / half)).astype(
        np.float32
    )
    freqs = np.concatenate([freqs, freqs], axis=0)
    t = np.arange(grid, dtype=np.float32)
    f = np.outer(t, freqs).astype(np.float32)
    fh = np.broadcast_to(f[:, None, :], (grid, grid, half))
    fw = np.broadcast_to(f[None, :, :], (grid, grid, half))
    full = np.concatenate([fh, fw], axis=-1).reshape(-1, HD).astype(np.float32)
    return np.cos(full).astype(np.float32), np.sin(full).astype(np.float32)


def _make_inputs(x, qkv_w, qkv_b, proj_w, proj_b, q_gamma, k_gamma,
                 use_bias=False):
    cos, sin = _rope_tables()
    sgn = np.where(np.arange(HD) < HD // 2, -1.0, 1.0).astype(np.float32)
    swap = (np.arange(HD) + HD // 2) % HD

    def fold(gamma, scale):
        c = (cos * gamma[None, :] * scale).astype(np.float32)
        s = (sin * sgn[None, :] * gamma[swap][None, :] * scale).astype(np.float32)
        return c, s

    cq, sq = fold(q_gamma.astype(np.float32), QSCALE)
    ck, sk = fold(k_gamma.astype(np.float32), 1.0)
    stack = [cq, sq, ck, sk]
    tabs = np.stack(stack, axis=0).reshape(4, NT, 128, HD).astype(NPBF16)

    ws = (qkv_w.astype(np.float32) * WSCALE).reshape(KT, 128, 3 * C)
    whi = np.ascontiguousarray(ws).astype(NPF8)
    wlo = (ws - whi.astype(np.float32)).astype(NPF8)
    wproj_h = np.ascontiguousarray(
        (proj_w.astype(np.float32) / WSCALE).reshape(KT, 128, C)
    ).astype(NPBF16)

    in_maps = []
    for c in range(N_CORES):
        xc = x[c * BSH : (c + 1) * BSH].astype(np.float32)  # [BSH, N, C]
        xt = np.ascontiguousarray(xc.transpose(0, 2, 1)).reshape(BSH, KT, 128, N)
        xhi = xt.astype(NPF8)
        m = {
            "xhi": xhi,
            "whi": whi,
            "wlo": wlo,
            "wproj": wproj_h,
            "tabs": tabs,
        }
        if use_bias:
            m["bq"] = (qkv_b.astype(np.float32) * WSCALE).astype(NPBF16)
            m["bp"] = proj_b.astype(np.float32).astype(NPBF16)
        in_maps.append(m)
    return in_maps


def _run(in_maps, use_bias=False, trace=False, **kwargs):
    nc = _get_nc(use_bias)
    return run_bass_kernel_spmd(
        nc, in_maps, core_ids=list(range(N_CORES)), trace=trace, **kwargs
    )


def kernel(x, qkv_w, qkv_b, proj_w, proj_b, q_gamma, k_gamma):
    x = np.asarray(x)
    qkv_b = np.asarray(qkv_b)
    proj_b = np.asarray(proj_b)
    use_bias = bool(np.any(qkv_b != 0) or np.any(proj_b != 0))
    q_gamma = np.asarray(q_gamma)
    k_gamma = np.asarray(k_gamma)
    in_maps = _make_inputs(
        x, np.asarray(qkv_w), qkv_b, np.asarray(proj_w), proj_b,
        q_gamma, k_gamma, use_bias=use_bias,
    )
    res = _run(in_maps, use_bias=use_bias)
    outs = [res.results[c]["out"].reshape(BSH, NT * 128, C) for c in range(N_CORES)]
    return np.concatenate(outs, axis=0).astype(np.float32)


# revision 11
# speedup vs baseline: 1.1938x; 1.0753x over previous
"""Trainium2 Bass kernel for nn_Attention_57827439673725.

Dense transformer attention block (B=32, N=1024, C=1024, H=16, hd=64):
  qkv = x @ qkv_w + qkv_b ; q,k rms-normed (per head) and 2D-roped;
  out = softmax(q k^T / sqrt(hd)) v @ proj_w + proj_b

Pure data-parallel over batch across 8 NeuronCores (4 batches each).

v4 design (fp8 attention core; exp split across ACT+DVE; rope tail on Pool):
  phase A (per token tile): qkv = 2-chain fp8e4 DoubleRow product
        xhi*(whi+wlo) (the xlo*whi chain of v2/v3 is dropped: its error is
        attention-averaged to ~0.2% final; halves the x DMA). q/k psums are
        ACT-copied to SBUF bf16, rms-normed + roped on DVE; the two final
        rope ops run on the otherwise-idle Pool engine and emit fp8e4
        directly (q scaled x8 in the host tables for fp8 range). The fp8
        q|k tile [128, 2048] is XBAR-transposed as uint16 PAIRS: head h
        lands at partitions 32*(h%4)..+32 with the (d, d+1) hd-pair in the
        two bytes of each u16 - exactly the [32, 2(pair), tok] layout that
        DoubleRow needs to contract hd=64 as 2x32 partition-tiles at 0.5
        cycles/row. v lands via one ACT copy into fp8 v65 ([v_h|1] per
        head).
  phase B (per head h, per j-tile): S^T = k q^T in fp8 DR (half the bf16
        cost); exp over the [128 j, 1024 i] psum emits P^T fp8e4 into a
        [128, 2(jt), 1024] pair tile, alternating engines: ACT slots use
        table exp (scale 1/64, bias -2 so max fp8 448 is never hit), DVE
        slots use a 1-op Schraudolph bit-trick (bits = psum*0.18034 +
        32.467 -> saturating round-to-nearest uint8 = the fp8e4m3 bit
        pattern; low end saturates to +0.0, exactly softmax semantics).
        O accumulates per jt-PAIR in DR mode: stationary P^T pair
        [128, 2, 128], moving v65 pair [128, 2, 64] -> 32.5 cycles per
        128x64 output (4x cheaper than v3); the softmax denominator runs
        as a parallel DR matmul against an fp8 ones pair. normalize =
        per-partition DVE reciprocal + one broadcast-last multiply.
  phase C: attn4 -> DMA XBAR transpose -> attnT; proj bf16; psum -> SBUF;
        DMA out fp32.

Slot pipeline: same weave as v2/v3 (C(b-1) + A(b+1) generators yield
between chunks of PE work inside B(b) slots) - phase B's PE work is now
tiny (~360ns/slot) vs the exp engines (~1.1us/slot), so the weave is what
keeps PE busy and ramped. PSUM: one shared pool of 3x [128,1024] f32
(2 banks each: S slots, qkv tiles, proj tiles) + psO [128, NT, 64] +
psD [128, H, NT] = 8 banks.
"""

import os
import sys

import numpy as np

for _p in ("/opt/trn_rl_repo",):
    if os.path.isdir(_p) and _p not in sys.path:
        sys.path.insert(0, _p)

import ml_dtypes  # noqa: E402

import concourse.bass as bass  # noqa: E402
import concourse.mybir as mybir  # noqa: E402
import concourse.tile as tile  # noqa: E402
from concourse import bacc  # noqa: E402
from concourse.bass_utils import run_bass_kernel_spmd  # noqa: E402

BF16 = mybir.dt.bfloat16
F32 = mybir.dt.float32
F8 = mybir.dt.float8e4
U16 = mybir.dt.uint16
U8 = mybir.dt.uint8
NPBF16 = ml_dtypes.bfloat16
NPF8 = ml_dtypes.float8_e4m3fn

N_CORES = 8
B, N, C = 32, 1024, 1024
H, HD = 16, 64
BSH = B // N_CORES  # batches per core
NT = N // 128  # token tiles per batch
KT = C // 128  # k tiles over C
KP = KT // 2  # fp8 DoubleRow k-pair count
EPS = 1e-06
THETA = 10000.0
WSCALE = 32.0  # qkv_w prescale (clears fp8e4m3 subnormals)
QSCALE = 8.0  # q rope-table prescale (fp8 range for S operands)

# exp: logit L = S_psum/64 (q carries x8); P = exp(L - DELTA) in fp8e4m3.
DELTA = 2.0
EXP_S1 = 8.0 / (64.0 * np.log(2.0))  # 0.18033688
EXP_S2 = 56.0 - 0.45 - DELTA * 8.0 / np.log(2.0)  # 32.467 (c=-0.45 tuned)

MULT = mybir.AluOpType.mult
ADD = mybir.AluOpType.add
DR = mybir.MatmulPerfMode.DoubleRow
CHUNK = int(os.environ.get("KCHUNK", "4"))
PROLOG = int(os.environ.get("KPROLOG", "2"))
EXPDVE = int(os.environ.get("KEXPDVE", "3"))  # of every 8 slots, this many on DVE
O_DELAY = int(os.environ.get("KODELAY", "2"))  # in jt-pairs
KPOOL = int(os.environ.get("KPOOL", "7"))  # 1: rope add on Pool; 2: t13; 4: qs-mul


def _ap_with(ap: bass.AP, dims) -> bass.AP:
    return bass.AP(tensor=ap.tensor, offset=ap.offset, ap=dims)


def _bcast_mid(ap: bass.AP, n: int) -> bass.AP:
    """[P, F] -> [P, n, F] with a 0-step broadcast middle dim."""
    return _ap_with(ap, [ap.ap[0], [0, n], *ap.ap[1:]])


def _bcast_last(ap: bass.AP, n: int) -> bass.AP:
    """[P, F] -> [P, F, n] with a 0-step broadcast last dim."""
    return _ap_with(ap, [*ap.ap, [0, n]])


def _build_module(use_bias: bool):
    nc = bacc.Bacc(
        "TRN2", target_bir_lowering=False, debug=False,
        dynamic_dma_scratch_size=2048,
    )

    xhi_d = nc.dram_tensor("xhi", [BSH, KT, 128, N], F8, kind="ExternalInput")
    whi_d = nc.dram_tensor("whi", [KT, 128, 3 * C], F8, kind="ExternalInput")
    wlo_d = nc.dram_tensor("wlo", [KT, 128, 3 * C], F8, kind="ExternalInput")
    wproj_d = nc.dram_tensor("wproj", [KT, 128, C], BF16, kind="ExternalInput")
    tabs_d = nc.dram_tensor("tabs", [4, NT, 128, HD], BF16, kind="ExternalInput")
    if use_bias:
        bq_d = nc.dram_tensor("bq", [3 * C], BF16, kind="ExternalInput")  # *WSCALE
        bp_d = nc.dram_tensor("bp", [C], BF16, kind="ExternalInput")
    out_d = nc.dram_tensor("out", [BSH, NT, 128, C], F32, kind="ExternalOutput")

    from contextlib import ExitStack

    with ExitStack() as ctx:
        tc = ctx.enter_context(tile.TileContext(nc))
        pool = lambda name, bufs, **kw: ctx.enter_context(  # noqa: E731
            tc.tile_pool(name=name, bufs=bufs, **kw)
        )
        cfg = dict(
            qk=2, v65=2, pt=4, at=2, rope=2, qs=2, t1=2, t2=2, stats=2,
            rec=2, outs=2, psA=3, psO=1,
        )
        for kv in os.environ.get("KBUFS", "").split(","):
            if kv:
                kk, vv = kv.split("=")
                cfg[kk] = int(vv)

        wpool = pool("weights", 1)
        cpool = pool("consts", 1)
        xtpool = pool("xt", 1)
        qkpool = pool("qkT", cfg["qk"])
        vpool = pool("v65", cfg["v65"])
        ptpool = pool("pt", cfg["pt"])
        a4pool = pool("attn4", 1)
        atpool = pool("attnT", cfg["at"])
        rpool = pool("rope", cfg["rope"])
        qspool = pool("qs", cfg["qs"])
        t1pool = pool("t1", cfg["t1"])
        t2pool = pool("t2", cfg["t2"])
        spool = pool("stats", cfg["stats"])
        recpool = pool("rec", cfg["rec"])
        opool = pool("outs", cfg["outs"])
        psA = pool("psA", cfg["psA"], space="PSUM")
        psO2 = pool("psO", cfg["psO"], space="PSUM")
        psDp = pool("psD", 1, space="PSUM")

        # ---- persistent weights / constants ----
        whi = wpool.tile([128, KT, 3 * C], F8, tag="whi")
        wlo = wpool.tile([128, KT, 3 * C], F8, tag="wlo")
        wproj = wpool.tile([128, KT, C], BF16, tag="wproj")

        tabs = cpool.tile([128, 4, NT, HD], BF16, tag="tabs")
        for i in range(4):
            nc.sync.dma_start(
                out=tabs[:, i, :, :], in_=tabs_d[i].rearrange("t p d -> p t d")
            )
        if use_bias:
            bias_qkv = cpool.tile([128, 3 * C], BF16, tag="bq")
            bq_ap = bq_d[:]
            nc.sync.dma_start(
                out=bias_qkv[:, :], in_=_ap_with(bq_ap, [[0, 128], *bq_ap.ap])
            )
            bias_proj = cpool.tile([128, C], BF16, tag="bp")
            bp_ap = bp_d[:]
            nc.sync.dma_start(
                out=bias_proj[:, :], in_=_ap_with(bp_ap, [[0, 128], *bp_ap.ap])
            )
        ones2 = cpool.tile([128, 2, 1], F8, tag="ones2")
        nc.vector.memset(ones2[:, :, :], 1.0)
        negd = cpool.tile([128, 1], F32, tag="negd")
        nc.vector.memset(negd[:, :], -DELTA)

        def load_x(b):
            xhi = xtpool.tile([128, KT, N], F8, tag="xhi", name="xhi")
            nc.sync.dma_start(
                out=xhi[:, :, :], in_=xhi_d[b].rearrange("k p n -> p k n")
            )
            return xhi

        def load_weights(b0):
            # first-consumed slices first: the opening q-chain needs only
            # whi/wlo k-pair cols 0:512 and x tile 0 - land those in small
            # DMAs so PE starts early, then stream the rest
            nc.sync.dma_start(
                out=whi[:, :, 0:512],
                in_=whi_d[:, :, 0:512].rearrange("k p n -> p k n"),
            )
            xhi = xtpool.tile([128, KT, N], F8, tag="xhi", name="xhi")
            nc.sync.dma_start(
                out=xhi[:, :, 0:128], in_=xhi_d[b0, :, :, 0:128].rearrange("k p n -> p k n")
            )
            nc.sync.dma_start(
                out=wlo[:, :, 0:512],
                in_=wlo_d[:, :, 0:512].rearrange("k p n -> p k n"),
            )
            nc.sync.dma_start(
                out=whi[:, :, 512:1024],
                in_=whi_d[:, :, 512:1024].rearrange("k p n -> p k n"),
            )
            nc.sync.dma_start(
                out=wlo[:, :, 512:1024],
                in_=wlo_d[:, :, 512:1024].rearrange("k p n -> p k n"),
            )
            nc.sync.dma_start(
                out=xhi[:, :, 128:], in_=xhi_d[b0, :, :, 128:].rearrange("k p n -> p k n")
            )
            nc.sync.dma_start(
                out=whi[:, :, 1024:],
                in_=whi_d[:, :, 1024:].rearrange("k p n -> p k n"),
            )
            nc.sync.dma_start(
                out=wlo[:, :, 1024:],
                in_=wlo_d[:, :, 1024:].rearrange("k p n -> p k n"),
            )
            nc.sync.dma_start(
                out=wproj[:, :, :], in_=wproj_d[:, :, :].rearrange("k p n -> p k n")
            )
            return xhi

        def qkv_mm(ps, xt, col_lo, chunk=None):
            """2-chain fp8 DR product into ps[:, 0:1024].

            Generator when chunk is set: yields between groups of `chunk`
            matmuls so the caller can interleave other PE work.
            """
            chains = [whi, wlo]
            nch = len(chains)
            emitted = 0
            for ci, ww in enumerate(chains):
                for kp in range(KP):
                    for half in range(2):
                        nc.tensor.matmul(
                            ps[:, half * 512 : (half + 1) * 512],
                            xt[:, 2 * kp : 2 * kp + 2, :],
                            ww[:, 2 * kp : 2 * kp + 2,
                               col_lo + half * 512 : col_lo + (half + 1) * 512],
                            start=(ci == 0 and kp == 0),
                            stop=(ci == nch - 1 and kp == KP - 1),
                            perf_mode=DR,
                        )
                        emitted += 1
                        if chunk and emitted % chunk == 0:
                            yield

        def qk_pipeline(stage, qi, t, qrope):
            """rms norm + rope for q (qi=0) or k (qi=1). stage is the bf16
            SBUF copy of the qkv psum. Writes fp8 into qrope[:, qi*1024:]."""
            src = stage
            t1 = t1pool.tile([128, 1024], BF16, tag="t1", name="t1")
            sq = t1[:, :]
            nc.vector.tensor_mul(sq, src[:, :], src[:, :])
            sq3 = sq.rearrange("p (h d) -> p h d", d=HD)
            nc.vector.tensor_add(sq3[:, :, 0:32], sq3[:, :, 0:32], sq3[:, :, 32:64])
            var = spool.tile([128, H], F32, tag="var", name="var")
            nc.vector.reduce_sum(
                var[:, :], sq3[:, :, 0:32], axis=mybir.AxisListType.X
            )
            # rsqrt(var/HD + eps): DVE ln-bit-trick + exp-bit-trick + Newton
            vv = spool.tile([128, H], F32, tag="vv", name="vv")
            nc.vector.tensor_scalar(
                out=vv[:, :], in0=var[:, :], scalar1=1.0 / HD, scalar2=EPS,
                op0=MULT, op1=ADD,
            )
            lnv = spool.tile([128, H], F32, tag="lnv", name="lnv")
            nc.vector.tensor_scalar(
                out=lnv[:, :], in0=vv[:, :].bitcast(mybir.dt.int32),
                scalar1=-1064866805, scalar2=8.2629582e-8,
                op0=ADD, op1=MULT,
            )
            r0i = spool.tile([128, H], mybir.dt.int32, tag="r0", name="r0i")
            nc.vector.tensor_scalar(
                out=r0i[:, :], in0=lnv[:, :],
                scalar1=-6051101.6, scalar2=1064866805.0,
                op0=MULT, op1=ADD,
            )
            r0 = r0i[:, :].bitcast(mybir.dt.float32)
            rr = spool.tile([128, H], F32, tag="rr", name="rr")
            e2 = spool.tile([128, H], F32, tag="e2", name="e2")
            cur = r0
            for _ in range(2):
                nc.vector.tensor_mul(e2[:, :], cur, cur)
                nc.vector.scalar_tensor_tensor(
                    out=e2[:, :], in0=e2[:, :], scalar=-0.5, in1=vv[:, :],
                    op0=MULT, op1=MULT,
                )
                nc.vector.scalar_tensor_tensor(
                    out=rr[:, :], in0=e2[:, :], scalar=1.5, in1=cur,
                    op0=ADD, op1=MULT,
                )
                cur = rr[:, :]

            # qs = src * rr (in-place into stage; bcast_last breaks the DVE
            # 4x mode so this is a 1x op - park it on the idle Pool engine)
            qs3 = src[:, :].rearrange("p (h d) -> p h d", d=HD)
            engqs = nc.gpsimd if (KPOOL & 4) else nc.vector
            engqs.tensor_mul(qs3, qs3, _bcast_last(rr[:, :], HD))

            # rope: out = qs*C + swap_halves(qs)*S; final ops emit fp8
            ctab = tabs[:, 2 * qi + 0, t, :]
            stab = tabs[:, 2 * qi + 1, t, :]
            t13 = t1[:, :].rearrange("p (h d) -> p h d", d=HD)
            eng13 = nc.gpsimd if (KPOOL & 2) else nc.vector
            eng13.tensor_mul(
                t13[:, :, 0:32], qs3[:, :, 32:64], _bcast_mid(stab[:, 0:32], H)
            )
            eng13.tensor_mul(
                t13[:, :, 32:64], qs3[:, :, 0:32], _bcast_mid(stab[:, 32:64], H)
            )
            t2 = t2pool.tile([128, 1024], BF16, tag="t2", name="t2")
            nc.vector.tensor_mul(
                t2[:, :].rearrange("p (h d) -> p h d", d=HD), qs3,
                _bcast_mid(ctab, H),
            )
            dst = qrope[:, qi * 1024 : (qi + 1) * 1024]
            engadd = nc.gpsimd if (KPOOL & 1) else nc.vector
            engadd.tensor_add(dst, t2[:, :], t1[:, :])

        def a_step_gen(xt, qkT2, v65, t, chunk=CHUNK):
            """one token tile of phase A as a generator: yields between PE
            chunks so the caller can interleave B-phase slots."""
            qrope = rpool.tile([128, 2 * C], F8, tag="qrope", name="qrope")
            xts = xt[:, :, t * 128 : (t + 1) * 128]
            stages = []
            for qi in range(2):
                ph = psA.tile([128, 1024], F32, tag="A", name="ps_qk")
                yield from qkv_mm(ph[:, :], xts, qi * 1024, chunk=chunk)
                stage = qspool.tile([128, 1024], BF16, tag="stage", name="stage")
                if use_bias:
                    nc.vector.scalar_tensor_tensor(
                        out=stage[:, :], in0=ph[:, :], scalar=1.0,
                        in1=bias_qkv[:, qi * 1024 : (qi + 1) * 1024],
                        op0=MULT, op1=ADD,
                    )
                else:
                    nc.scalar.copy(stage[:, :], ph[:, :])
                stages.append(stage)
                if qi == 1:
                    qk_pipeline(stages[0], 0, t, qrope)
                yield

            # v: 2-chain DR into one [128, 1024] psum; one ACT copy into v65
            psv = psA.tile([128, 1024], F32, tag="A", name="psv")
            yield from qkv_mm(psv[:, :], xts, 2048, chunk=chunk)
            v3 = v65[:, t, :].rearrange("p (h e) -> p h e", e=65)
            pv3 = psv[:, :].rearrange("p (h d) -> p h d", d=64)
            if use_bias:
                nc.vector.scalar_tensor_tensor(
                    out=v3[:, :, 0:64], in0=pv3, scalar=1.0,
                    in1=bias_qkv[:, 2048:3072].rearrange("p (h d) -> p h d", d=64),
                    op0=MULT, op1=ADD,
                )
            else:
                nc.scalar.copy(v3[:, :, 0:64], pv3)
            qk_pipeline(stages[1], 1, t, qrope)
            yield
            # q|k rope fp8 [128, 2048] -> u16-pair transpose -> qkT2
            nc.sync.dma_start_transpose(
                qkT2[:, :, t * 128 : (t + 1) * 128], qrope[:, :].bitcast(U16)
            )

        def v65_init(v65):
            v4 = v65[:, :, :].rearrange("p t (h e) -> p t h e", e=65)
            nc.vector.memset(v4[:, :, :, 64:65], 1.0)

        def b_phase(attn4, qkT2, v65, weave_gen):
            """slot-pipelined phase B: per (h, jt) slot emit S(DR)+exp; per
            jt-pair emit the delayed O-DR octet."""
            pending = []  # (h, jp, pt)
            psos = {}
            psd = psDp.tile([128, H, NT], F32, tag="D", name="psd")
            slot = 0

            def emit_o(h, jp, pt):
                if jp == 0:
                    psos[h] = psO2.tile([128, NT, 64], F32, tag="O", name="ps_o")
                ps_o = psos[h]
                vsl = v65[:, 2 * jp : 2 * jp + 2, h * 65 : h * 65 + 64]
                for ib in range(NT):
                    ptb = pt[:, :, ib * 128 : (ib + 1) * 128]
                    nc.tensor.matmul(
                        ps_o[:, ib, :], ptb, vsl,
                        start=(jp == 0 and ib == 0),
                        stop=(jp == 3 and ib == NT - 1),
                        perf_mode=DR,
                        skip_group_check=True,
                    )
                    nc.tensor.matmul(
                        psd[:, h, ib : ib + 1], ptb, ones2[:, :, :],
                        start=(h == 0 and jp == 0 and ib == 0),
                        stop=(h == H - 1 and jp == 3 and ib == NT - 1),
                        perf_mode=DR,
                        skip_group_check=True,
                    )
                if jp == 3:
                    rec = recpool.tile([128, NT], BF16, tag="rec", name="rec")
                    with nc.allow_low_precision("softmax denom recip bf16"):
                        nc.vector.reciprocal(rec[:, :], psd[:, h, :])
                    nc.vector.tensor_mul(
                        attn4[:, :, h, :], psos[h][:, :, :],
                        _bcast_last(rec[:, :], 64),
                    )
                    del psos[h]

            pt = None
            for h in range(H):
                pb = 32 * (h % 4)
                fbq = h // 4
                fbk = 4 + h // 4
                for jt in range(NT):
                    if weave_gen is not None:
                        next(weave_gen, None)
                    if jt % 2 == 0:
                        pt = ptpool.tile([128, 2, 1024], F8, tag="pt", name="pt")
                    ps_s = psA.tile([128, 1024], F32, tag="A", name="ps_s")
                    lk = qkT2[pb : pb + 32, fbk, jt * 128 : (jt + 1) * 128] \
                        .bitcast(F8).rearrange("p (t b) -> p b t", b=2)
                    for ic in range(2):
                        rq = qkT2[pb : pb + 32, fbq, ic * 512 : (ic + 1) * 512] \
                            .bitcast(F8).rearrange("p (i b) -> p b i", b=2)
                        nc.tensor.matmul(
                            ps_s[:, ic * 512 : (ic + 1) * 512],
                            lk, rq, start=True, stop=True, perf_mode=DR,
                            tile_position=(pb, 0),
                        )
                    if len(pending) >= O_DELAY:
                        emit_o(*pending.pop(0))
                    dst = pt[:, jt % 2, :]
                    if (slot * EXPDVE) % 8 < EXPDVE:
                        nc.vector.tensor_scalar(
                            out=dst.bitcast(U8), in0=ps_s[:, :],
                            scalar1=EXP_S1, scalar2=EXP_S2,
                            op0=MULT, op1=ADD,
                        )
                    else:
                        nc.scalar.activation(
                            dst, ps_s[:, :],
                            mybir.ActivationFunctionType.Exp,
                            scale=1.0 / 64.0, bias=negd[:, :],
                        )
                    slot += 1
                    if jt % 2 == 1:
                        pending.append((h, jt // 2, pt))
            for args in pending:
                emit_o(*args)

        def c_gen(attn4, b):
            """phase C as a generator (woven into the next batch's B slots):
            attn4 -> XBAR transpose -> attnT -> proj -> DMA out."""

            def transpose(t):
                att = atpool.tile([128, KT, 128], BF16, tag="at", name="attnT")
                nc.sync.dma_start_transpose(att[:, :, :], attn4[:, t, :, :])
                return att

            att = transpose(0)
            yield
            for t in range(NT):
                att_next = transpose(t + 1) if t + 1 < NT else None
                ps_p = psA.tile([128, 1024], F32, tag="A", name="ps_p")
                for half in range(2):
                    for k in range(KT):
                        nc.tensor.matmul(
                            ps_p[:, half * 512 : (half + 1) * 512],
                            att[:, k, :],
                            wproj[:, k, half * 512 : (half + 1) * 512],
                            start=(k == 0), stop=(k == KT - 1),
                        )
                    yield
                ostage = opool.tile([128, C], F32, tag="ostage", name="ostage")
                if use_bias:
                    nc.vector.tensor_add(
                        ostage[:, :], ps_p[:, :], bias_proj[:, :]
                    )
                elif os.environ.get("KOCOPY", "split") == "act":
                    nc.scalar.copy(ostage[:, :], ps_p[:, :])
                elif os.environ.get("KOCOPY", "split") == "dve":
                    nc.vector.tensor_copy(ostage[:, :], ps_p[:, :])
                else:
                    nc.scalar.copy(ostage[:, 0:512], ps_p[:, 0:512])
                    nc.vector.tensor_copy(ostage[:, 512:1024], ps_p[:, 512:1024])
                nc.sync.dma_start(out=out_d[b, t], in_=ostage[:, :])
                yield
                att = att_next

        def alloc_ab():
            qkT2 = qkpool.tile([128, KT, N], U16, tag="qkT2", name="qkT2")
            v65 = vpool.tile([128, NT, H * 65], F8, tag="v65", name="v65")
            return qkT2, v65

        reps = int(os.environ.get("KREPEAT", "1"))
        batches = [bb for _ in range(reps) for bb in range(BSH)]

        # prologue: weights + A(b0), two token-tile pipelines interleaved
        xt = load_weights(batches[0])
        tiles = alloc_ab()
        v65_init(tiles[1])
        from collections import deque

        _done = object()
        gens = [a_step_gen(xt, tiles[0], tiles[1], t) for t in range(NT)]
        active = deque(gens[:PROLOG])
        gi = PROLOG
        while active:
            g = active.popleft()
            if next(g, _done) is not _done:
                active.append(g)
            elif gi < NT:
                active.append(gens[gi])
                gi += 1

        from itertools import chain as _ichain

        prev_c = None  # (attn4, b) awaiting phase C
        for bi, b in enumerate(batches):
            qkT2, v65 = tiles
            attn4 = a4pool.tile([128, NT, H, HD], BF16, tag="attn4", name="attn4")
            nxt = batches[bi + 1] if bi + 1 < len(batches) else None
            wparts = []
            if prev_c is not None:
                wparts.append(c_gen(*prev_c))
            if nxt is not None:
                xt2 = load_x(nxt)
                tiles2 = alloc_ab()
                v65_init(tiles2[1])

                _ck = int(os.environ.get("KCHUNK0", "3")) if prev_c is None else CHUNK

                def _weave(_xt=xt2, _tl=tiles2, _ck=_ck):
                    for t in range(NT):
                        yield from a_step_gen(_xt, _tl[0], _tl[1], t, chunk=_ck)

                wparts.append(_weave())
            wg = _ichain(*wparts) if wparts else None
            b_phase(attn4, qkT2, v65, wg)
            if wg is not None:
                for _ in wg:
                    pass
            prev_c = (attn4, b)
            if nxt is not None:
                xt, tiles = xt2, tiles2
        for _ in c_gen(*prev_c):
            pass

    nc.compile()
    return nc


_NC = {}


def _get_nc(use_bias: bool = False, share_tabs: bool = False):
    key = bool(use_bias)
    if key not in _NC:
        _NC[key] = _build_module(key)
    return _NC[key]


def _rope_tables():
    """cos/sin tables exactly as reference.rope_tables, in float32."""
    grid = int(np.sqrt(N))
    half = HD // 2
    freqs = (1.0 / THETA ** (np.arange(0, half, 2, dtype=np.float32) / half)).astype(
        np.float32
    )
    freqs = np.concatenate([freqs, freqs], axis=0)
    t = np.arange(grid, dtype=np.float32)
    f = np.outer(t, freqs).astype(np.float32)
    fh = np.broadcast_to(f[:, None, :], (grid, grid, half))
    fw = np.broadcast_to(f[None, :, :], (grid, grid, half))
    full = np.concatenate([fh, fw], axis=-1).reshape(-1, HD).astype(np.float32)
    return np.cos(full).astype(np.float32), np.sin(full).astype(np.float32)


def _make_inputs(x, qkv_w, qkv_b, proj_w, proj_b, q_gamma, k_gamma,
                 use_bias=False):
    cos, sin = _rope_tables()
    sgn = np.where(np.arange(HD) < HD // 2, -1.0, 1.0).astype(np.float32)
    swap = (np.arange(HD) + HD // 2) % HD

    def fold(gamma, scale):
        c = (cos * gamma[None, :] * scale).astype(np.float32)
        s = (sin * sgn[None, :] * gamma[swap][None, :] * scale).astype(np.float32)
        return c, s

    cq, sq = fold(q_gamma.astype(np.float32), QSCALE)
    ck, sk = fold(k_gamma.astype(np.float32), 1.0)
    stack = [cq, sq, ck, sk]
    tabs = np.stack(stack, axis=0).reshape(4, NT, 128, HD).astype(NPBF16)

    ws = (qkv_w.astype(np.float32) * WSCALE).reshape(KT, 128, 3 * C)
    whi = np.ascontiguousarray(ws).astype(NPF8)
    wlo = (ws - whi.astype(np.float32)).astype(NPF8)
    wproj_h = np.ascontiguousarray(
        (proj_w.astype(np.float32) / WSCALE).reshape(KT, 128, C)
    ).astype(NPBF16)

    in_maps = []
    for c in range(N_CORES):
        xc = x[c * BSH : (c + 1) * BSH].astype(np.float32)  # [BSH, N, C]
        xt = np.ascontiguousarray(xc.transpose(0, 2, 1)).reshape(BSH, KT, 128, N)
        xhi = xt.astype(NPF8)
        m = {
            "xhi": xhi,
            "whi": whi,
            "wlo": wlo,
            "wproj": wproj_h,
            "tabs": tabs,
        }
        if use_bias:
            m["bq"] = (qkv_b.astype(np.float32) * WSCALE).astype(NPBF16)
            m["bp"] = proj_b.astype(np.float32).astype(NPBF16)
        in_maps.append(m)
    return in_maps


def _run(in_maps, use_bias=False, trace=False, **kwargs):
    nc = _get_nc(use_bias)
    return run_bass_kernel_spmd(
        nc, in_maps, core_ids=list(range(N_CORES)), trace=trace, **kwargs
    )


def kernel(x, qkv_w, qkv_b, proj_w, proj_b, q_gamma, k_gamma):
    x = np.asarray(x)
    qkv_b = np.asarray(qkv_b)
    proj_b = np.asarray(proj_b)
    use_bias = bool(np.any(qkv_b != 0) or np.any(proj_b != 0))
    q_gamma = np.asarray(q_gamma)
    k_gamma = np.asarray(k_gamma)
    in_maps = _make_inputs(
        x, np.asarray(qkv_w), qkv_b, np.asarray(proj_w), proj_b,
        q_gamma, k_gamma, use_bias=use_bias,
    )
    res = _run(in_maps, use_bias=use_bias)
    outs = [res.results[c]["out"].reshape(BSH, NT * 128, C) for c in range(N_CORES)]
    return np.concatenate(outs, axis=0).astype(np.float32)
